# revision 14
# baseline (speedup 1.0000x reference)
"""TRN2 Bass kernel for nn_BTGINs (2-layer GIN message passing), 8 NeuronCores.

Design (SPMD — one program, per-core data):
- Host relabels nodes into "slots": 8 cores x TPC tiles x 128 slots,
  bin-packed so per-tile in-edge counts are balanced; output is unpermuted on
  the host. Both layers share the same graph, so gather indices / dst-slot
  arrays are computed once.
- Messages are gathered node-major ([128 msgs/chunk] on partitions, 256B bf16
  rows) with the custom SWDGE dma_gather (int16 idxs -> 4 table buckets of
  25088 rows), rotated across 4 SWDGE queues (latency-bound: ~2.9ns/row).
- Aggregation: one-hot S [128 msgs, 256 dst] built on DVE via
  is_equal(dstloc, iota256); PE matmul accumulates agg_fm [128 feat, 256 dst]
  over the chunks of a 2-tile window. Padded messages carry dstloc=300 which
  matches no iota column (zero contribution).
- MLP/BN in feature-major layout; BN batch stats via a tiny AllReduce of
  per-feature (sum, sumsq); the linear bias before BN cancels and is dropped.
- Layer-1 output tiles are PE-transposed to node-major and AllGathered into a
  bf16 [NSLOT, 128] table for layer-2 gathers.
"""

import math
import numpy as np
import ml_dtypes

import concourse.bass as bass
import concourse.bacc as bacc
import concourse.mybir as mybir
import concourse.tile as tile
from concourse import bass_utils, library_config

F = 128
P = 128
NCORES = 8
NBUCK = 4
BN_EPS = 1e-5
PAD_DLOC = 300.0  # not in [0, 256) -> S row all zero

N_FULL = 100000
TPC_FULL = 98  # tiles/core; 98*128*8 = 100352 slots >= 100000


# ----------------------------------------------------------------------------
# host-side prep
# ----------------------------------------------------------------------------

def _binpack(deg, ntiles):
    import heapq

    n = len(deg)
    node_of_slot = np.full(ntiles * P, -1, np.int64)
    slot_of_node = np.empty(n, np.int64)
    tile_cnt = np.zeros(ntiles, np.int32)
    tile_load = np.zeros(ntiles, np.int64)
    heap = [(0, t) for t in range(ntiles)]
    heapq.heapify(heap)
    for v in np.argsort(-deg, kind="stable"):
        while True:
            load, t = heapq.heappop(heap)
            if tile_cnt[t] < P:
                break
        pos = tile_cnt[t]
        tile_cnt[t] = pos + 1
        tile_load[t] = load + deg[v]
        node_of_slot[t * P + pos] = v
        slot_of_node[v] = t * P + pos
        if tile_cnt[t] < P:
            heapq.heappush(heap, (int(tile_load[t]), t))
    return slot_of_node, node_of_slot


def _prep(x, src, dst, eps1, tpc):
    n = x.shape[0]
    ntiles = NCORES * tpc
    nslot = ntiles * P
    spc = tpc * P
    assert spc % NBUCK == 0 and nslot % NBUCK == 0
    bsz = nslot // NBUCK  # bucket size (rows); must be < 32768
    assert bsz < 32768

    deg = np.bincount(dst, minlength=n)
    slot_of_node, node_of_slot = _binpack(deg, ntiles)

    sdst = slot_of_node[dst]
    ssrc = slot_of_node[src]

    nb2 = (tpc + 1) // 2  # batch2 = 2-tile window (256 dst slots)
    # sort edges by (core, batch2, bucket)
    core = sdst // spc
    b2 = (sdst % spc) // (2 * P)
    buck = ssrc // bsz
    key = (core * nb2 + b2) * NBUCK + buck
    order = np.argsort(key, kind="stable")
    e_key = key[order]
    e_lidx = (ssrc % bsz)[order]  # idx within bucket table
    e_dloc = (sdst % (2 * P))[order]  # dst offset within 2-tile window

    cnt = np.bincount(e_key, minlength=NCORES * nb2 * NBUCK).reshape(
        NCORES, nb2, NBUCK
    )
    quota = np.ceil(cnt.max(axis=0) / P).astype(np.int64)  # [nb2, NBUCK] chunks
    starts = np.zeros(NCORES * nb2 * NBUCK, np.int64)
    np.cumsum(cnt.reshape(-1)[:-1], out=starts[1:])

    # slot layout: ggroup = 4 consecutive batch2 windows; within a ggroup,
    # bucket-major: [b0: p0..p3 | b1: p0..p3 | ...]; chunk positions global.
    ngg = (nb2 + 3) // 4
    gg_of = np.arange(nb2) // 4
    chunk_pos = np.zeros((nb2, NBUCK), np.int64)  # first chunk slot of (p,b)
    call_info = []  # per ggroup: [(bucket, chunk_start, n_chunks)]
    pos = 0
    for g in range(ngg):
        ps = [p for p in range(4 * g, min(4 * g + 4, nb2))]
        calls = []
        for b in range(NBUCK):
            c0 = pos
            for p in ps:
                chunk_pos[p, b] = pos
                pos += quota[p, b]
            if pos > c0:
                calls.append((b, c0, pos - c0))
        call_info.append(calls)
    total_chunks = pos

    # fill idx / dloc arrays (chunk-slot layout; message m of chunk c ->
    # partition m, column c)
    idx_arr = np.zeros((NCORES, total_chunks * P), np.int64)  # local bucket idx
    dloc_arr = np.full((NCORES, total_chunks * P), PAD_DLOC, np.float64)
    for c in range(NCORES):
        for p in range(nb2):
            for b in range(NBUCK):
                k = (c * nb2 + p) * NBUCK + b
                cc = cnt[c, p, b]
                s = starts[k]
                base = chunk_pos[p, b] * P
                idx_arr[c, base : base + cc] = e_lidx[s : s + cc]
                dloc_arr[c, base : base + cc] = e_dloc[s : s + cc]

    # idxs SBUF image: per gather call, wrap its message list into 16
    # partitions, replicate to 128. Calls are contiguous chunk ranges.
    idx_sb = np.zeros((NCORES, P, total_chunks * 8), np.int16)
    for g in range(ngg):
        for (b, c0, nch) in call_info[g]:
            nmsg = nch * P
            for c in range(NCORES):
                lst = idx_arr[c, c0 * P : c0 * P + nmsg]
                w = lst.reshape(nmsg // 16, 16).T  # [16, nmsg/16]
                idx_sb[c, :, c0 * 8 : c0 * 8 + nmsg // 16] = np.tile(
                    w, (8, 1)
                ).astype(np.int16)

    # dloc SBUF image [128, total_chunks] bf16: column c = chunk c
    dloc_sb = (
        dloc_arr.reshape(NCORES, total_chunks, P)
        .transpose(0, 2, 1)
        .astype(ml_dtypes.bfloat16)
    )

    x_slot = np.zeros((nslot, F), np.float32)
    m = node_of_slot >= 0
    x_slot[m] = x[node_of_slot[m]]
    x_tab = x_slot.astype(ml_dtypes.bfloat16)
    xs = (1.0 + float(eps1)) * x_slot
    x_own = xs.reshape(NCORES, spc, F).transpose(0, 2, 1).astype(ml_dtypes.bfloat16)

    # per-batch2 chunk schedule: list of chunk slots (column in msg buffer /
    # dloc), grouped per batch2 in position order
    sched = []
    for p in range(nb2):
        cols = []
        for b in range(NBUCK):
            cols.extend(range(chunk_pos[p, b], chunk_pos[p, b] + quota[p, b]))
        sched.append(sorted(cols))

    return dict(
        node_of_slot=node_of_slot,
        nslot=nslot,
        spc=spc,
        bsz=bsz,
        nb2=nb2,
        ngg=ngg,
        call_info=call_info,
        total_chunks=total_chunks,
        sched=sched,
        idx_sb=idx_sb,
        dloc_sb=dloc_sb,
        x_tab=x_tab,
        x_own=x_own,
    )


# ----------------------------------------------------------------------------
# device program
# ----------------------------------------------------------------------------

def _build(tpc, pr, eps2, n_bn, no_collectives=False, core0=0, stage='full'):
    BF = mybir.dt.bfloat16
    FP = mybir.dt.float32
    spc = tpc * P
    nslot = NCORES * spc
    nb2 = pr["nb2"]
    ngg = pr["ngg"]
    call_info = pr["call_info"]
    sched = pr["sched"]
    total_chunks = pr["total_chunks"]
    bsz = pr["bsz"]
    rg = [list(range(NCORES))]
    W2 = 2 * P  # S width / batch2 dst window

    max_gg_chunks = max(
        sum(n for (_, _, n) in call_info[g]) for g in range(ngg)
    )

    nc = bacc.Bacc(
        "TRN2", target_bir_lowering=False, debug=False, num_swdge_queues=4
    )

    x_tab = nc.declare_dram_parameter("x_tab", [nslot, F], BF, isOutput=False)
    idxs = nc.declare_dram_parameter(
        "idxs", [P, total_chunks * 8], mybir.dt.int16, isOutput=False
    )
    dlocs = nc.declare_dram_parameter("dlocs", [P, total_chunks], BF, isOutput=False)
    x_own = nc.declare_dram_parameter("x_own", [P, spc], BF, isOutput=False)
    w1a = nc.declare_dram_parameter("w1a", [F, F], BF, isOutput=False)
    w1b = nc.declare_dram_parameter("w1b", [F, F], BF, isOutput=False)
    w2a = nc.declare_dram_parameter("w2a", [F, F], BF, isOutput=False)
    w2b = nc.declare_dram_parameter("w2b", [F, F], BF, isOutput=False)
    vecs = nc.declare_dram_parameter("vecs", [P, 6], FP, isOutput=False)
    iota = nc.declare_dram_parameter("iota", [P, W2], BF, isOutput=False)
    ident = nc.declare_dram_parameter("ident", [P, P], BF, isOutput=False)
    identf = nc.declare_dram_parameter("identf", [P, P], FP, isOutput=False)
    out_ext = nc.declare_dram_parameter("out", [spc, F], FP, isOutput=True)

    h_shard = nc.dram_tensor("h_shard", [spc, F], BF)
    h_tab = nc.dram_tensor("h_tab", [nslot, F], BF)
    bn_in = nc.dram_tensor("bn_in", [P, 2], FP)
    bn_out = nc.dram_tensor("bn_out", [P, 2], FP)

    with tile.TileContext(nc) as tc:
        import contextlib

        with contextlib.ExitStack() as ctx:
            singles = ctx.enter_context(tc.tile_pool(name="singles", bufs=1))
            msgs_p = ctx.enter_context(tc.tile_pool(name="msgs", bufs=2))
            s_p = ctx.enter_context(tc.tile_pool(name="s", bufs=8))
            h0_p = ctx.enter_context(tc.tile_pool(name="h0", bufs=3))
            own_p = ctx.enter_context(tc.tile_pool(name="own", bufs=3))
            sc_p = ctx.enter_context(tc.tile_pool(name="scratch", bufs=2))
            trs_p = ctx.enter_context(tc.tile_pool(name="trs", bufs=4))
            vec_p = ctx.enter_context(tc.tile_pool(name="vec", bufs=2))
            aggp = ctx.enter_context(tc.tile_pool(name="aggp", bufs=2, space="PSUM"))
            mlpp = ctx.enter_context(tc.tile_pool(name="mlpp", bufs=2, space="PSUM"))
            trp = ctx.enter_context(tc.tile_pool(name="trp", bufs=2, space="PSUM"))

            nc.gpsimd.load_library(library_config.mlp)

            sb_idx = singles.tile([P, total_chunks * 8], mybir.dt.int16)
            nc.sync.dma_start(out=sb_idx[:], in_=idxs[:])
            sb_dloc = singles.tile([P, total_chunks], BF)
            nc.sync.dma_start(out=sb_dloc[:], in_=dlocs[:])
            sb_w = {}
            for nm, t in (("w1a", w1a), ("w1b", w1b), ("w2a", w2a), ("w2b", w2b)):
                sb_w[nm] = singles.tile([F, F], BF, tag=f"sb_{nm}", name=f"sb_{nm}")
                nc.sync.dma_start(out=sb_w[nm][:], in_=t[:])
            sb_iota = singles.tile([P, W2], BF)
            nc.sync.dma_start(out=sb_iota[:], in_=iota[:])
            sb_ident = singles.tile([P, P], BF)
            nc.sync.dma_start(out=sb_ident[:], in_=ident[:])
            sb_identf = singles.tile([P, P], FP)
            nc.sync.dma_start(out=sb_identf[:], in_=identf[:])
            sb_vecs = singles.tile([P, 6], FP)
            nc.sync.dma_start(out=sb_vecs[:], in_=vecs[:])

            sb_eps = singles.tile([P, 1], FP)
            nc.vector.memset(sb_eps[:], BN_EPS)
            sb_h1m = singles.tile([P, spc], BF)
            sb_hl1 = singles.tile([P, spc], BF)
            sb_stat = singles.tile([P, 2 * nb2], FP)
            if stage != "full":
                nc.vector.memset(sb_h1m[:], 0.0)
                nc.vector.memset(sb_hl1[:], 0.0)
                nc.vector.memset(sb_stat[:], 0.0)

            qrot = [0]

            def layer(li):
                tab = x_tab if li == 0 else h_tab
                wa = sb_w["w1a" if li == 0 else "w2a"]
                wb = sb_w["w1b" if li == 0 else "w2b"]
                o = 0 if li == 0 else 3
                g_ap = sb_vecs[:, o : o + 1]
                bt_ap = sb_vecs[:, o + 1 : o + 2]
                bb_ap = sb_vecs[:, o + 2 : o + 3]

                # ---- phase 1 ----
                for g in range(ngg):
                    calls = call_info[g]
                    g_c0 = calls[0][1]
                    g_chunks = sum(n for (_, _, n) in calls)
                    msgs = msgs_p.tile([P, max_gg_chunks, F], BF, tag="msgs")
                    for (b, c0, nch) in calls:
                        nmsg = nch * P
                        nc.gpsimd.dma_gather(
                            msgs[:, c0 - g_c0 : c0 - g_c0 + nch, :],
                            tab[b * bsz : (b + 1) * bsz, :],
                            sb_idx[:, c0 * 8 : c0 * 8 + nmsg // 16],
                            nmsg,
                            nmsg,
                            F,
                            single_packet=False,
                            queue_num=qrot[0] % 4,
                        )
                        qrot[0] += 1
                    if stage == "g0":
                        continue
                    for p in range(4 * g, min(4 * g + 4, nb2)):
                        ncol = min(W2, spc - p * W2)
                        agg = aggp.tile([P, W2], FP, tag="agg")
                        cols = sched[p]
                        for j, cpos in enumerate(cols):
                            S = s_p.tile([P, W2], BF, tag="S")
                            nc.vector.tensor_tensor(
                                out=S[:],
                                in0=sb_dloc[:, cpos : cpos + 1].to_broadcast([P, W2]),
                                in1=sb_iota[:],
                                op=mybir.AluOpType.is_equal,
                            )
                            if stage == "s1":
                                continue
                            nc.tensor.matmul(
                                agg[:],
                                lhsT=msgs[:, cpos - g_c0, :],
                                rhs=S[:],
                                start=(j == 0),
                                stop=(j == len(cols) - 1),
                            )
                        if stage in ("s1", "s2"):
                            continue
                        h0 = h0_p.tile([P, W2], BF, tag="h0")
                        if li == 0:
                            own = own_p.tile([P, W2], BF, tag="own")
                            nc.sync.dma_start(
                                out=own[:, :ncol],
                                in_=x_own[:, p * W2 : p * W2 + ncol],
                            )
                        else:
                            own = own_p.tile([P, W2], BF, tag="own")
                            nc.scalar.activation(
                                out=own[:, :ncol],
                                in_=sb_hl1[:, p * W2 : p * W2 + ncol],
                                func=mybir.ActivationFunctionType.Copy,
                                scale=float(1.0 + eps2),
                            )
                        nc.vector.tensor_tensor(
                            out=h0[:, :ncol],
                            in0=agg[:, :ncol],
                            in1=own[:, :ncol],
                            op=mybir.AluOpType.add,
                        )
                        if stage == "s3":
                            continue
                        h1m = mlpp.tile([P, W2], FP, space="PSUM", tag="mlp")
                        nc.tensor.matmul(
                            h1m[:, :ncol], lhsT=wa[:], rhs=h0[:, :ncol],
                            start=True, stop=True,
                        )
                        if stage == "s4":
                            continue
                        nc.scalar.activation(
                            out=sb_h1m[:, p * W2 : p * W2 + ncol],
                            in_=h1m[:, :ncol],
                            func=mybir.ActivationFunctionType.Copy,
                            accum_out=sb_stat[:, 2 * p : 2 * p + 1],
                        )
                        if stage == "s5":
                            continue
                        sq = sc_p.tile([P, W2], BF, tag="sq")
                        nc.scalar.activation(
                            out=sq[:, :ncol],
                            in_=h1m[:, :ncol],
                            func=mybir.ActivationFunctionType.Square,
                            accum_out=sb_stat[:, 2 * p + 1 : 2 * p + 2],
                        )

                if stage in ("g0", "p1"):
                    return
                # ---- BN stats ----
                stat2 = vec_p.tile([P, 2], FP, tag="stat2")
                nc.vector.reduce_sum(
                    out=stat2[:],
                    in_=sb_stat[:].rearrange("p (b two) -> p two b", two=2),
                    axis=mybir.AxisListType.X,
                )
                nc.sync.dma_start(out=bn_in[:], in_=stat2[:])
                if no_collectives:
                    nc.sync.dma_start(out=bn_out[:], in_=bn_in[:])
                else:
                    nc.gpsimd.collective_compute(
                        "AllReduce",
                        mybir.AluOpType.add,
                        replica_groups=rg,
                        ins=[bn_in.ap().opt()],
                        outs=[bn_out.ap().opt()],
                    )
                sb_bn = vec_p.tile([P, 2], FP, tag="sb_bn")
                nc.sync.dma_start(out=sb_bn[:], in_=bn_out[:])

                mu = vec_p.tile([P, 1], FP, tag="mu")
                nc.vector.tensor_scalar_mul(mu[:], sb_bn[:, 0:1], 1.0 / n_bn)
                var = vec_p.tile([P, 1], FP, tag="var")
                nc.vector.tensor_scalar_mul(var[:], sb_bn[:, 1:2], 1.0 / n_bn)
                mu2 = vec_p.tile([P, 1], FP, tag="mu2")
                nc.vector.tensor_tensor(
                    out=mu2[:], in0=mu[:], in1=mu[:], op=mybir.AluOpType.mult
                )
                nc.vector.tensor_tensor(
                    out=var[:], in0=var[:], in1=mu2[:], op=mybir.AluOpType.subtract
                )
                sd = vec_p.tile([P, 1], FP, tag="sd")
                nc.scalar.activation(
                    out=sd[:], in_=var[:],
                    func=mybir.ActivationFunctionType.Sqrt, bias=sb_eps[:],
                )
                rinv = vec_p.tile([P, 1], FP, tag="rinv")
                nc.vector.reciprocal(rinv[:], sd[:])
                a_ap = vec_p.tile([P, 1], FP, tag="a")
                nc.vector.tensor_tensor(
                    out=a_ap[:], in0=rinv[:], in1=g_ap, op=mybir.AluOpType.mult
                )
                c_ap = vec_p.tile([P, 1], FP, tag="c")
                nc.vector.tensor_tensor(
                    out=c_ap[:], in0=mu[:], in1=a_ap[:], op=mybir.AluOpType.mult
                )
                nc.vector.tensor_tensor(
                    out=c_ap[:], in0=bt_ap, in1=c_ap[:], op=mybir.AluOpType.subtract
                )

                if stage == "bn":
                    return
                # ---- phase 2 ----
                for p in range(nb2):
                    ncol = min(W2, spc - p * W2)
                    h1n = h0_p.tile([P, W2], BF, tag="h1n")
                    nc.scalar.activation(
                        out=h1n[:, :ncol],
                        in_=sb_h1m[:, p * W2 : p * W2 + ncol],
                        func=mybir.ActivationFunctionType.Relu,
                        scale=a_ap[:],
                        bias=c_ap[:],
                    )
                    h2 = mlpp.tile([P, W2], FP, space="PSUM", tag="mlp")
                    nc.tensor.matmul(
                        h2[:, :ncol], lhsT=wb[:], rhs=h1n[:, :ncol],
                        start=True, stop=True,
                    )
                    if li == 0:
                        nc.scalar.activation(
                            out=sb_hl1[:, p * W2 : p * W2 + ncol],
                            in_=h2[:, :ncol],
                            func=mybir.ActivationFunctionType.Relu,
                            bias=bb_ap,
                        )
                        for tt in range(ncol // P):
                            t = 2 * p + tt
                            trp_t = trp.tile([P, P], BF, space="PSUM", tag="trp")
                            nc.tensor.transpose(
                                out=trp_t[:],
                                in_=sb_hl1[:, t * P : (t + 1) * P],
                                identity=sb_ident[:],
                            )
                            trs = trs_p.tile([P, P], BF, tag="trs")
                            nc.scalar.activation(
                                out=trs[:], in_=trp_t[:],
                                func=mybir.ActivationFunctionType.Copy,
                            )
                            nc.sync.dma_start(
                                out=h_shard[t * P : (t + 1) * P, :], in_=trs[:]
                            )
                    else:
                        of32 = sc_p.tile([P, W2], FP, tag="of32")
                        nc.vector.tensor_tensor(
                            out=of32[:, :ncol],
                            in0=h2[:, :ncol],
                            in1=bb_ap.to_broadcast([P, ncol]),
                            op=mybir.AluOpType.add,
                        )
                        for tt in range(ncol // P):
                            t = 2 * p + tt
                            trp_t = trp.tile([P, P], FP, space="PSUM", tag="trp")
                            nc.tensor.transpose(
                                out=trp_t[:],
                                in_=of32[:, tt * P : (tt + 1) * P],
                                identity=sb_identf[:],
                            )
                            trs = trs_p.tile([P, P], FP, tag="trsf")
                            nc.scalar.activation(
                                out=trs[:], in_=trp_t[:],
                                func=mybir.ActivationFunctionType.Copy,
                            )
                            nc.sync.dma_start(
                                out=out_ext[t * P : (t + 1) * P, :], in_=trs[:]
                            )

                if stage == "p2":
                    return
                if li == 0:
                    if no_collectives:
                        nc.sync.dma_start(
                            out=h_tab[core0 * spc : (core0 + 1) * spc, :],
                            in_=h_shard[:],
                        )
                    else:
                        nc.gpsimd.collective_compute(
                            "AllGather",
                            mybir.AluOpType.bypass,
                            replica_groups=rg,
                            ins=[h_shard.ap().opt()],
                            outs=[h_tab.ap().opt()],
                        )

            layer(0)
            if stage == "full":
                layer(1)

    nc.compile()
    return nc


# ----------------------------------------------------------------------------
# entry
# ----------------------------------------------------------------------------

def _make_inputs(pr, inputs, tpc):
    bfl = ml_dtypes.bfloat16
    W2 = 2 * P
    vecs = np.stack(
        [
            np.asarray(inputs["g1"], np.float32),
            np.asarray(inputs["bt1"], np.float32),
            np.asarray(inputs["b1b"], np.float32),
            np.asarray(inputs["g2"], np.float32),
            np.asarray(inputs["bt2"], np.float32),
            np.asarray(inputs["b2b"], np.float32),
        ],
        axis=1,
    )
    iota = np.tile(np.arange(W2, dtype=np.float32), (P, 1)).astype(bfl)
    ident = np.eye(P, dtype=np.float32).astype(bfl)
    identf = np.eye(P, dtype=np.float32)
    w = {
        k: np.asarray(inputs[k], np.float32).astype(bfl)
        for k in ("w1a", "w1b", "w2a", "w2b")
    }
    in_maps = []
    for c in range(NCORES):
        in_maps.append(
            dict(
                x_tab=pr["x_tab"],
                idxs=pr["idx_sb"][c],
                dlocs=pr["dloc_sb"][c],
                x_own=pr["x_own"][c],
                vecs=vecs, iota=iota, ident=ident, identf=identf, **w,
            )
        )
    return in_maps


def _run(inputs, tpc, n_bn, trace=False):
    x = np.asarray(inputs["x"], np.float32)
    src = np.asarray(inputs["src"], np.int64)
    dst = np.asarray(inputs["dst"], np.int64)
    eps1 = float(np.asarray(inputs["eps1"]))
    eps2 = float(np.asarray(inputs["eps2"]))

    pr = _prep(x, src, dst, eps1, tpc)
    nc = _build(tpc, pr, eps2, n_bn)
    in_maps = _make_inputs(pr, inputs, tpc)
    res = bass_utils.run_bass_kernel_spmd(
        nc, in_maps, list(range(NCORES)), trace=trace
    )
    outs = [np.asarray(res.results[c]["out"], np.float32) for c in range(NCORES)]
    out_slot = np.concatenate(outs, axis=0)
    nos = pr["node_of_slot"]
    m = nos >= 0
    out = np.zeros((x.shape[0], F), np.float32)
    out[nos[m]] = out_slot[m]
    if trace:
        return out, res
    return out


def kernel(**inputs) -> np.ndarray:
    return _run(inputs, TPC_FULL, N_FULL)
